# revision 42
# baseline (speedup 1.0000x reference)
"""Trainium2 Bass kernel for nn_Attention_17334488007364.

Computation (per batch element, x as [C=128, N=4096]):
    q = wq @ x                      [16, 4096]
    k = maxpool2(wk @ x)            [16, 1024]
    v = maxpool2(wv @ x)            [64, 1024]
    attn = softmax(q^T k, axis=m)   [4096, 1024]
    o = v @ attn^T                  [64, 4096]
    out = gamma * (wo @ o) + x      [128, 4096]

Sharding: pure data parallel -- B=16 over 8 cores, 2 batch elements/core.

v3 design notes (evolved from the v1 all-bf16 kernel):
  - The scores matmul floor is one PE output column/cycle (8.4M fp32 PSUM
    scores per core = 65.5k cols); contraction tricks don't change that,
    so scores stay bf16 K=16 q-form, but emitted as 1024-wide moving
    operands (64 matmuls instead of 128 -- halves instruction overhead),
    with NO row-group packing and no q/k replication fleet: pooled k is
    copied once per batch to partitions 0:16 (one DMA per half) so lhsT
    and rhs share base partition 0.
  - The PE idles ~0.5us between score strips (ACT-paced), which keeps the
    HAM clock gate COLD (1.2GHz) for the entire run in v1/v2 (first HAM
    un-throttle event at ~100us / never). A ~20-matmul warmup spin on
    constant data, hidden under the initial x DMA loads, trips the
    activity window early so the whole kernel runs at 2.4GHz.
  - AV is fp8 DoubleRow (the one place DR genuinely halves PE work since
    the m=1024 contraction needed 8 accumulating bf16 matmuls): lhsT =
    vT~ pairs [128, 2, 65] e4m3 with slot stride 80 (pair step %16==0
    LDWEIGHTS rule), col 64 = ones so accumulator row 64 is the softmax
    denominator; rhs = two m-strips of p (e5m2) per instruction.
  - exp runs on ACT with bias -2.5 writing e5m2 directly (smax ~= 12 ->
    max p ~= 1.3e4 < 57344; row-max min ~= -1.2 keeps every row normal).
    Measured end-to-end rel err ~4e-4 (the residual add uses f32 x).
  - epilogue: gamma folded into woT; denominators DMA-packed [4, 512] ->
    f32 -> reciprocal_approx_fast -> bf16 -> DRAM round-trip broadcast;
    onorm = ou * rden as bf16 tensor_tensor; residual added in the output
    drain (tensor_tensor add with f32 x) instead of an identity matmul.
  - queues: all DMAs on sync; pools stage1 on GPSIMD; ACT only exps.
"""

from contextlib import ExitStack

import numpy as np

import concourse.bacc as bacc
import concourse.mybir as mybir
from concourse import masks
from concourse.alu_op_type import AluOpType
from concourse.tile import TileContext

FP32 = mybir.dt.float32
F32R = mybir.dt.float32r
BF16 = mybir.dt.bfloat16
FP8 = mybir.dt.float8e4      # e4m3
FP8W = mybir.dt.float8e5     # e5m2 (attention weights)
AFT = mybir.ActivationFunctionType
DR = mybir.MatmulPerfMode.DoubleRow

# Per-core problem shape (hardcoded; harness provides full inputs).
B_FULL, C, H, W = 16, 128, 64, 64
N_CORES = 8
B_LOC = B_FULL // N_CORES            # 2
HW = H * W                           # 4096
M = HW // 4                          # 1024 (after 2x2 maxpool)
CQ, CV = C // 8, C // 2              # 16, 64
NCH = 512                            # epilogue chunk (psum-bank n span)
NCHUNKS = HW // NCH                  # 8
NSP = 1024                           # score span (moving operand width)
NSPANS = HW // NSP                   # 4
MT = M // 128                        # 8 m-strips of 128

EXP_BIAS = -2.5                      # exp(s + EXP_BIAS) fits e5m2
VTS = 80                             # vT~ pair slot stride (16-aligned)
WARMUP_MM = 20                       # HAM warmup matmuls


def build_nc():
    nc = bacc.Bacc()
    x_e = nc.declare_dram_parameter("x", [B_LOC, C, HW], FP32, isOutput=False)
    wq_e = nc.declare_dram_parameter("wq", [CQ, C], FP32, isOutput=False)
    wk_e = nc.declare_dram_parameter("wk", [CQ, C], FP32, isOutput=False)
    wv_e = nc.declare_dram_parameter("wv", [CV, C], FP32, isOutput=False)
    wo_e = nc.declare_dram_parameter("wo", [C, CV], FP32, isOutput=False)
    g_e = nc.declare_dram_parameter("gamma", [1], FP32, isOutput=False)
    out_e = nc.declare_dram_parameter("out", [B_LOC, C, HW], FP32, isOutput=True)

    with TileContext(nc) as tc, ExitStack() as ctx:
        const = ctx.enter_context(tc.tile_pool(name="const", bufs=1))
        xpool = ctx.enter_context(tc.tile_pool(name="x", bufs=2))
        qkv = ctx.enter_context(tc.tile_pool(name="qkv", bufs=2))
        ppool = ctx.enter_context(tc.tile_pool(name="p", bufs=4))
        vtpool = ctx.enter_context(tc.tile_pool(name="vt", bufs=8))
        eppool = ctx.enter_context(tc.tile_pool(name="ep", bufs=3))
        outpool = ctx.enter_context(tc.tile_pool(name="outp", bufs=3))
        # PSUM budget (8 banks): scores 2 tags x 1 buf x 2 banks + av 2x1
        # + wm 2x1
        ps_s = ctx.enter_context(tc.tile_pool(name="ps_s", bufs=1, space="PSUM"))
        ps_av = ctx.enter_context(tc.tile_pool(name="ps_av", bufs=2, space="PSUM"))
        ps_w = ctx.enter_context(tc.tile_pool(name="ps_w", bufs=2, space="PSUM"))
        dscratch = ctx.enter_context(tc.tile_pool(name="dscr", bufs=4, space="DRAM"))

        # ---------------- constants / weight preprocessing ----------------
        ident = const.tile([128, 128], FP32)
        masks.make_identity(nc, ident[:])
        ident_bf = const.tile([128, 128], BF16)
        masks.make_identity(nc, ident_bf[:])

        wq_sb = const.tile([CQ, C], FP32, tag="wq")
        wk_sb = const.tile([CQ, C], FP32, tag="wk")
        wv_sb = const.tile([CV, C], FP32, tag="wv")
        wo_sb = const.tile([C, CV], FP32, tag="wo")
        nc.sync.dma_start(wq_sb[:], wq_e[:])
        nc.sync.dma_start(wk_sb[:], wk_e[:])
        nc.sync.dma_start(wv_sb[:], wv_e[:])
        nc.sync.dma_start(wo_sb[:], wo_e[:])

        # gamma broadcast to all 128 partitions: [128, 1]
        g_sb = const.tile([128, 1], FP32, tag="g")
        nc.sync.dma_start(
            g_sb[:, 0:1], g_e[:].unsqueeze(0).partition_broadcast(128)
        )

        # exp bias as an explicit per-partition scalar
        ebias = const.tile([128, 1], FP32, tag="ebias")
        nc.vector.memset(ebias[:], EXP_BIAS)

        # heater source for HAM full-array keep-warm matmuls
        heat_src = const.tile([128, NCH], BF16, tag="heat")
        nc.vector.memset(heat_src[:], 0.0)

        def heater(tag, n=4):
            # full-array matmuls on constant data: count as PE-busy for the
            # HAM activity monitor so the clock stays at 2.4GHz through the
            # tiled score packs (which do not count)
            hp = ps_w.tile([128, NCH], FP32, tag="wm", name=f"heat_{tag}")
            for hi in range(n):
                nc.tensor.matmul(
                    hp[:], ident_bf[:], heat_src[:], start=True, stop=True
                )

        # W_cat^T: cols 0:16 = wq^T, 32:48 = wk^T, 64:128 = wv^T
        ps_wt = ps_w.tile([128, NCH], FP32, tag="wm")
        nc.tensor.transpose(ps_wt[:, 0:CQ], wq_sb[:], ident[0:CQ, 0:CQ])
        nc.tensor.transpose(ps_wt[:, 32 : 32 + CQ], wk_sb[:], ident[0:CQ, 0:CQ])
        nc.tensor.transpose(ps_wt[:, 64 : 64 + CV], wv_sb[:], ident[0:CV, 0:CV])
        wcatT = const.tile([128, 128], BF16, tag="wcatT")
        nc.vector.memset(wcatT[:], 0.0)
        nc.vector.tensor_copy(wcatT[:, 0:CQ], ps_wt[:, 0:CQ])
        nc.vector.tensor_copy(wcatT[:, 32 : 32 + CQ], ps_wt[:, 32 : 32 + CQ])
        nc.vector.tensor_copy(wcatT[:, 64 : 64 + CV], ps_wt[:, 64 : 64 + CV])

        # woT [64, 128] bf16 with gamma folded in
        wog = const.tile([C, CV], FP32, tag="wog")
        nc.vector.tensor_scalar_mul(wog[:], wo_sb[:], g_sb[:, 0:1])
        ps_wo = ps_w.tile([128, NCH], FP32, tag="wm")
        nc.tensor.transpose(ps_wo[0:CV, 0:C], wog[:], ident[:])
        woT = const.tile([CV + 1, C], BF16, tag="woT")
        nc.vector.tensor_copy(woT[0:CV, :], ps_wo[0:CV, 0:C])
        nc.vector.memset(woT[CV : CV + 1, :], 0.0)

        # ---------------- per-batch prep ----------------

        def load_x(b, x_sb=None, chunks=range(NCHUNKS)):
            if x_sb is None:
                x_sb = xpool.tile([C, HW], FP32, tag="x", name=f"x_{b}")
            for cc in chunks:
                csl = slice(cc * NCH, (cc + 1) * NCH)
                nc.sync.dma_start(x_sb[:, csl], x_e[b, :, csl])
            return x_sb

        def prep_init(b, x_sb):
            return {
                "b": b,
                "x_sb": x_sb,
                "x_bf": qkv.tile([C, HW], BF16, tag="xbf", name=f"xbf_{b}"),
                "qv_full": qkv.tile([C, HW], BF16, tag="qvfull", name=f"qf_{b}"),
                "kv_sb": qkv.tile([128, M], BF16, tag="k", name=f"kv_{b}"),
                # q / pooled-k replicated to partitions {0,32,64,96}+0:16 so
                # 4 m-strips of scores run concurrently as 4 PE row groups
                "qrep": qkv.tile([128, HW], BF16, tag="qrep", name=f"qr_{b}"),
                "krep": qkv.tile([128, M], BF16, tag="krep", name=f"kr_{b}"),
                "vt8": [None] * 4,
            }

        def pool_rows(st, cc, lo, hi, eng=None):
            qv_full, kv_sb = st["qv_full"], st["kv_sb"]
            b = st["b"]
            if eng is None:
                eng = nc.vector
            sl = slice(cc * NCH, (cc + 1) * NCH)
            kv1 = qkv.tile([128, 4 * 64], BF16, tag="kv1",
                           name=f"kv1_{b}_{cc}_{lo}")
            pp = qv_full[lo:hi, sl].rearrange(
                "p (h2 two w) -> p h2 two w", h2=4, two=2, w=64
            )
            s1 = kv1[lo:hi, :].rearrange("p (h w) -> p h w", h=4, w=64)
            eng.tensor_tensor(
                s1, pp[:, :, 0, :], pp[:, :, 1, :], AluOpType.max
            )
            s1w = kv1[lo:hi, :].rearrange(
                "p (h w2 two) -> p h w2 two", h=4, w2=32, two=2
            )
            s2 = kv_sb[lo:hi, cc * 128 : (cc + 1) * 128].rearrange(
                "p (h w2) -> p h w2", h=4, w2=32
            )
            eng.tensor_tensor(
                s2, s1w[:, :, :, 0], s1w[:, :, :, 1], AluOpType.max
            )

        def prep_chunk(st, cc):
            b = st["b"]
            x_sb, x_bf = st["x_sb"], st["x_bf"]
            qv_full, kv_sb = st["qv_full"], st["kv_sb"]
            sl = slice(cc * NCH, (cc + 1) * NCH)
            head = b == 0 and cc < 4
            # ACT is idle until the first exp; split the head-critical
            # casts of batch 0's first chunks between ACT and DVE
            if head:
                nc.scalar.copy(x_bf[:, sl], x_sb[:, sl])
            else:
                nc.vector.tensor_copy(x_bf[:, sl], x_sb[:, sl])
            ps_p = ps_w.tile([128, NCH], FP32, tag="wm", name=f"pj_{b}_{cc}")
            # single full-array matmul: counts as PE-busy for the HAM clock
            # gate (tiled/col-split matmuls do not), keeping the PE at 2.4GHz
            nc.tensor.matmul(
                ps_p[:], wcatT[:], x_bf[:, sl], start=True, stop=True
            )
            nc.vector.tensor_copy(qv_full[:, sl], ps_p[:])
            # k-pools gate the early scores; batch 0's v-pools + transposes
            # are deferred past the whole k chain
            pool_rows(st, cc, 32, 32 + CQ)
            if b != 0:
                pool_rows(st, cc, 64, 128)
                emit_vt(st, cc)
            # q / pooled-k replication to the 4 row-group partition offsets,
            # once per half-batch (sync queue; x1 loads are deferred so the
            # head replication isn't stuck behind them)
            if cc in (3, 7):
                h = cc // 4
                qsl = slice(h * 4 * NCH, (h + 1) * 4 * NCH)
                msl = slice(h * 512, (h + 1) * 512)
                for gi in range(4):
                    nc.sync.dma_start(
                        st["qrep"][32 * gi : 32 * gi + CQ, qsl],
                        qv_full[0:CQ, qsl],
                    )
                    nc.sync.dma_start(
                        st["krep"][32 * gi : 32 * gi + CQ, msl],
                        kv_sb[32 : 32 + CQ, msl],
                    )
            # head cut: strip-quad 0 of spans 0-1 only needs q chunks 0-3 and
            # the first pooled-k half -- start the score/exp pipeline early
            if b == 0 and cc == 3:
                heater("wu0", n=10)
                for esp in range(2):
                    pc = ppool.tile([128, MT * NSP], FP8W, tag="pc",
                                    name=f"pc_0_{esp}")
                    st.setdefault("early_pc", {})[esp] = pc
                    scores_packs(st, esp, pc, quads=(0,))

        def emit_vt(st, j):
            # vT~ pair tiles [128, 2*VTS] e4m3; strip j -> pair j//2, slot
            # j%2 at cols 0:65 / VTS:VTS+65; col 64 & VTS+64 = ones
            b, kv_sb = st["b"], st["kv_sb"]
            u, s = j // 2, j % 2
            ps_t = ps_w.tile([128, 128], BF16, tag="wm", name=f"tp_{b}_{j}")
            nc.tensor.transpose(
                ps_t[:, 0:CV],
                kv_sb[64:128, j * 128 : (j + 1) * 128],
                ident_bf[64:128, 64:128],
            )
            if s == 0:
                st["vt8"][u] = vtpool.tile([128, 2 * VTS], FP8, tag="vt",
                                           name=f"vt_{b}_{u}")
            vt = st["vt8"][u]
            off = s * VTS
            nc.vector.tensor_copy(vt[:, off : off + CV], ps_t[:, 0:CV])
            nc.vector.memset(vt[:, off + CV : off + CV + 1], 1.0)

        def scores_packs(st, sp, pc, quads=(0, 1), fillers=None):
            # 16-tile packs: 4 m-strips x 4 col groups of [K=16, O=32, N=512]
            # matmuls run concurrently in 32x32 PE tiling mode (~547ns per
            # 4 strips at the cold clock). Each pack lands in two 2-bank
            # PSUM tiles (strips 0,1 / 2,3) so the two exp reads pipeline
            # against the next pack's writes.
            b = st["b"]
            qrep, krep = st["qrep"], st["krep"]
            pcw = pc[:].rearrange("p (t n) -> p t n", t=MT, n=NSP)
            for qd in quads:
                for hh in range(2):
                    nsl = slice(sp * NSP + hh * NCH, sp * NSP + (hh + 1) * NCH)
                    halves = [
                        ps_s.tile([128, 2 * NCH], FP32, tag=f"s{z}",
                                  name=f"s_{b}_{sp}_{qd}_{hh}_{z}")
                        for z in range(2)
                    ]
                    for i in range(4):
                        t = 4 * qd + i
                        s_ps = halves[i // 2]
                        ioff = (i % 2) * NCH
                        for j in range(4):
                            nc.tensor.matmul(
                                s_ps[32 * j : 32 * (j + 1),
                                     ioff : ioff + NCH],
                                krep[32 * i : 32 * i + CQ,
                                     t * 128 + 32 * j : t * 128 + 32 * (j + 1)],
                                qrep[32 * i : 32 * i + CQ, nsl],
                                start=True,
                                stop=True,
                                tile_position=(32 * i, 32 * j),
                            )
                    for z in range(2):
                        nc.scalar.activation(
                            pcw[:, 4 * qd + 2 * z : 4 * qd + 2 * z + 2,
                                hh * NCH : (hh + 1) * NCH],
                            halves[z][:].rearrange(
                                "p (i n) -> p i n", i=2, n=NCH
                            ),
                            AFT.Exp,
                            bias=ebias[:, 0:1],
                        )
                    if fillers:
                        for _ in range(2):
                            if fillers:
                                fillers.pop(0)()

        def av_pass(st, ep, c, pc, u, reuse=False):
            # one DoubleRow accumulation pass (m-strips 2u, 2u+1) of chunk c
            b, vt8 = st["b"], st["vt8"]
            h = c % 2
            pcv = pc[:].rearrange("p (a two n) -> p a two n", a=4, two=2, n=NSP)
            if u == 0:
                ep.setdefault("o_ps", {})[c % 2] = ps_av.tile(
                    [CV + 1, NCH], FP32, tag="av", name=f"av_{b}_{c}"
                )
            mm = nc.tensor.matmul(
                ep["o_ps"][c % 2][:],
                _pair(vt8[u], VTS, 0, CV + 1),
                pcv[:, u, :, h * NCH : (h + 1) * NCH],
                start=(u == 0),
                stop=(u == 3),
                perf_mode=DR,
            )
            if reuse:
                mm.ins.ldweights = False

        def av_finish(st, ep, c):
            b = st["b"]
            ou = eppool.tile([CV + 1, NCH], BF16, tag="ou", bufs=9,
                             name=f"ou_{b}_{c}")
            nc.vector.tensor_copy(ou[:], ep["o_ps"][c % 2][:])
            ep["ou"][c % 4] = ou
            if ep.get("last"):
                # final group: per-chunk reciprocal chains (no 4-chunk
                # batching latency, one-hop gpsimd broadcast) so the tail
                # epilogues start as soon as each AV lands
                if "den" not in ep:
                    ep["den"] = eppool.tile([CV + 1, 4 * NCH], BF16, tag="den",
                                            bufs=2, name=f"den_{ep['bh'][0]}_L")
                dsc = eppool.tile([1, NCH], BF16, tag="dstL", bufs=4,
                                  name=f"dstL_{b}_{c}")
                nc.sync.dma_start(dsc[0:1, :], ou[CV : CV + 1, :])
                ds32 = eppool.tile([1, NCH], FP32, tag="ds32L", bufs=4,
                                   name=f"ds32L_{b}_{c}")
                nc.vector.tensor_copy(ds32[:], dsc[:])
                rdn = eppool.tile([1, NCH], FP32, tag="rdnL", bufs=4,
                                  name=f"rdnL_{b}_{c}")
                nc.vector.reciprocal_approx_fast(rdn[:], ds32[:])
                rdnb = eppool.tile([1, NCH], BF16, tag="rdnbL", bufs=4,
                                   name=f"rdnbL_{b}_{c}")
                nc.vector.tensor_copy(rdnb[:], rdn[:])
                nc.gpsimd.partition_broadcast(
                    ep["den"][:, (c % 4) * NCH : (c % 4 + 1) * NCH],
                    rdnb[0:1, :],
                )
            else:
                nc.sync.dma_start(
                    ep["dstage"][c % 4 : c % 4 + 1, :], ou[CV : CV + 1, :]
                )

        def den_chain(ep, lo=0, hi=4):
            # reciprocal + broadcast of dstage rows [lo, hi) (split for the
            # final group so its first chunks' epilogues don't wait on the
            # last chunk's AV). The hi==4 split part uses dedicated base-0
            # tiles (engine ops can't start at partition 3).
            b, h = ep["bh"]
            if "den" not in ep:
                ep["den"] = eppool.tile([CV + 1, 4 * NCH], BF16, tag="den", bufs=2,
                                        name=f"den_{b}_{h}")
            if lo == 0:
                n, src = hi, ep["dstage"]
            else:
                n, src = 1, ep["dstage2"]
            ds32 = eppool.tile([n, NCH], FP32, tag=f"ds32{lo}", bufs=2,
                               name=f"ds32_{b}_{h}_{lo}")
            nc.vector.tensor_copy(ds32[:], src[0:n, :])
            rdn = eppool.tile([n, NCH], FP32, tag=f"rdn{lo}", bufs=2,
                              name=f"rdn_{b}_{h}_{lo}")
            nc.vector.reciprocal_approx_fast(rdn[:], ds32[:])
            rdnb = eppool.tile([n, NCH], BF16, tag=f"rdnb{lo}", bufs=2,
                               name=f"rdnb_{b}_{h}_{lo}")
            nc.vector.tensor_copy(rdnb[:], rdn[:])
            rd4 = dscratch.tile([n, NCH], BF16, tag=f"rd{lo}",
                                name=f"rd4_{b}_{h}_{lo}")
            nc.sync.dma_start(rd4[:], rdnb[:])
            nc.sync.dma_start(
                ep["den"][:, lo * NCH : hi * NCH].rearrange(
                    "p (c n) -> p c n", c=hi - lo, n=NCH
                ),
                rd4[:].partition_broadcast(CV + 1),
            )

        def epilogue_chunk(st, ep, c):
            b, h = ep["bh"]
            x_sb = st["x_sb"]
            sl = slice(c * NCH, (c + 1) * NCH)
            onorm = eppool.tile([CV + 1, NCH], BF16, tag="onorm", bufs=3,
                                name=f"on_{b}_{c}")
            nc.vector.tensor_tensor(
                onorm[:],
                ep["ou"][c % 4][0 : CV + 1, :],
                ep["den"][:, (c % 4) * NCH : (c % 4 + 1) * NCH],
                AluOpType.mult,
            )
            o2_ps = ps_w.tile([128, NCH], FP32, tag="wm", name=f"o2_{b}_{c}")
            # K=65 rounds the PE tile mode up to full-array (HAM-counted);
            # woT row 64 is zero so the junk denominator row contributes 0
            nc.tensor.matmul(
                o2_ps[:], woT[:], onorm[:], start=True, stop=True
            )
            out_sb = outpool.tile([C, NCH], FP32, tag="out", name=f"os_{b}_{c}")
            nc.vector.tensor_tensor(
                out_sb[:], o2_ps[:], x_sb[:, sl], AluOpType.add
            )
            nc.sync.dma_start(out_e[b, :, sl], out_sb[:])

        # ---- emission: 8 span-iterations (4 spans x 2 batches) ----
        x0 = load_x(0)
        x1 = xpool.tile([C, HW], FP32, tag="x", name="x_1")
        st0 = prep_init(0, x0)
        st1 = prep_init(1, x1)
        for cc in range(NCHUNKS):
            prep_chunk(st0, cc)
        # x1 loads issued after b0's prep DMAs so the head qrep/krep
        # replication isn't queued behind them
        load_x(1, x_sb=x1)
        # batch 0's deferred v-pools + vT transposes (needed by AV(b0)
        # from span-iteration 1 onward)
        for cc in range(NCHUNKS):
            pool_rows(st0, cc, 64, 128)
            emit_vt(st0, cc)
        sts = {0: st0, 1: st1}

        av_q = []      # (st, ep, sp, pc) spans awaiting AV
        ep_q = []      # (st, ep, c, min_iter) awaiting epilogue
        eps = {}

        def av_fillers(pst, pep, psp, ppc, it):
            # AV for span psp as ONE consecutive 8-MM DoubleRow burst with a
            # heater tail: a sustained full-array block trips the HAM
            # activity window so the following packs run at 2.4GHz
            fs = []

            def fburst():
                for u in range(4):
                    av_pass(pst, pep, 2 * psp, ppc, u)
                    av_pass(pst, pep, 2 * psp + 1, ppc, u, reuse=True)
                heater(f"hb_{pst['b']}_{psp}", n=4)
            fs.append(fburst)

            def fin():
                for cch in (2 * psp, 2 * psp + 1):
                    av_finish(pst, pep, cch)
                    if pep.get("last"):
                        ep_q.append((pst, pep, cch, it))
                if not pep.get("last") and (2 * psp + 1) % 4 == 3:
                    den_chain(pep)
                    for ec in range(2 * psp - 2, 2 * psp + 2):
                        ep_q.append((pst, pep, ec, it + 1))
            fs.append(fin)
            return fs

        for i in range(2 * NSPANS):
            b, sp = i // NSPANS, i % NSPANS
            st = sts[b]
            g = (b, sp // 2)
            if g not in eps:
                eps[g] = {
                    "bh": g,
                    "dstage": eppool.tile([4, NCH], BF16, tag="dstage", bufs=3,
                                          name=f"dst_{g[0]}_{g[1]}"),
                    "ou": {},
                    "last": g == (1, 1),
                }
            fillers = []
            if i < NSPANS:
                fillers.append(lambda cc=2 * i: prep_chunk(st1, cc))
                fillers.append(lambda cc=2 * i + 1: prep_chunk(st1, cc))
            if i >= 1 and av_q:
                pst, pep, psp, ppc = av_q.pop(0)
                fillers.extend(av_fillers(pst, pep, psp, ppc, i))
            if b == 0 and sp in st.get("early_pc", {}):
                pc = st["early_pc"][sp]
                quads = (1,)
            else:
                pc = ppool.tile([128, MT * NSP], FP8W, tag="pc",
                                name=f"pc_{b}_{sp}")
                quads = (0, 1)
            if i == 2 * NSPANS - 1:
                # last span: its own AV interleaves into the span instead of
                # serializing into the tail
                fillers.extend(av_fillers(st, eps[g], sp, pc, i))
            scores_packs(st, sp, pc, quads=quads, fillers=fillers)
            for f in fillers:
                f()
            if i != 2 * NSPANS - 1:
                av_q.append((st, eps[g], sp, pc))
            npop = 0
            cap = 3 if i < 6 else 8
            while ep_q and ep_q[0][3] <= i and npop < cap:
                e = ep_q.pop(0)
                epilogue_chunk(e[0], e[1], e[2])
                npop += 1
        # drain remaining work; the final group computed per-chunk
        # denominators in av_finish so each epilogue follows its AV directly
        while av_q:
            pst, pep, psp, ppc = av_q.pop(0)
            for f in av_fillers(pst, pep, psp, ppc, 0):
                f()
            while ep_q:
                e = ep_q.pop(0)
                epilogue_chunk(e[0], e[1], e[2])
        while ep_q:
            e = ep_q.pop(0)
            epilogue_chunk(e[0], e[1], e[2])

    nc.finalize()
    return nc


def _pair(tile, slot, off, n):
    """DoubleRow k-pair view [P, 2, n] of a flat [P, 2*slot] tile: free dim
    is (two, slot) with the window [off:off+n] taken inside each slot."""
    return tile[:].rearrange("p (two f) -> p two f", two=2, f=slot)[
        :, :, off : off + n
    ]


_NC_CACHE = None


def _get_nc():
    global _NC_CACHE
    if _NC_CACHE is None:
        _NC_CACHE = build_nc()
    return _NC_CACHE


def kernel(**inputs) -> np.ndarray:
    from concourse.bass_utils import run_bass_kernel_spmd

    x = np.asarray(inputs["x"], dtype=np.float32).reshape(B_FULL, C, HW)
    wq = np.asarray(inputs["wq"], dtype=np.float32)
    wk = np.asarray(inputs["wk"], dtype=np.float32)
    wv = np.asarray(inputs["wv"], dtype=np.float32)
    wo = np.asarray(inputs["wo"], dtype=np.float32)
    gamma = np.asarray(inputs["gamma"], dtype=np.float32)

    nc = _get_nc()
    in_maps = []
    for i in range(N_CORES):
        in_maps.append(
            {
                "x": np.ascontiguousarray(x[i * B_LOC : (i + 1) * B_LOC]),
                "wq": wq,
                "wk": wk,
                "wv": wv,
                "wo": wo,
                "gamma": gamma,
            }
        )
    res = run_bass_kernel_spmd(nc, in_maps, core_ids=list(range(N_CORES)))
    outs = [res.results[i]["out"].reshape(B_LOC, C, H, W) for i in range(N_CORES)]
    return np.concatenate(outs, axis=0)


if __name__ == "__main__":
    import reference

    inputs = {k: np.asarray(v) for k, v in reference.setup_inputs().items()}
    expected = np.asarray(reference.reference(**inputs))
    actual = kernel(**inputs)
    err = np.linalg.norm(actual - expected) / np.linalg.norm(expected)
    print("Relative error:", err)



# revision 43
# speedup vs baseline: 1.2789x; 1.2789x over previous
"""Trainium2 Bass kernel for nn_Attention_17334488007364.

Computation (per batch element, x as [C=128, N=4096]):
    q = wq @ x                      [16, 4096]
    k = maxpool2(wk @ x)            [16, 1024]
    v = maxpool2(wv @ x)            [64, 1024]
    attn = softmax(q^T k, axis=m)   [4096, 1024]
    o = v @ attn^T                  [64, 4096]
    out = gamma * (wo @ o) + x      [128, 4096]

Sharding: pure data parallel -- B=16 over 8 cores, 2 batch elements/core.

v3 design notes (evolved from the v1 all-bf16 kernel):
  - The scores matmul floor is one PE output column/cycle (8.4M fp32 PSUM
    scores per core = 65.5k cols); contraction tricks don't change that,
    so scores stay bf16 K=16 q-form, but emitted as 1024-wide moving
    operands (64 matmuls instead of 128 -- halves instruction overhead),
    with NO row-group packing and no q/k replication fleet: pooled k is
    copied once per batch to partitions 0:16 (one DMA per half) so lhsT
    and rhs share base partition 0.
  - The PE idles ~0.5us between score strips (ACT-paced), which keeps the
    HAM clock gate COLD (1.2GHz) for the entire run in v1/v2 (first HAM
    un-throttle event at ~100us / never). A ~20-matmul warmup spin on
    constant data, hidden under the initial x DMA loads, trips the
    activity window early so the whole kernel runs at 2.4GHz.
  - AV is fp8 DoubleRow (the one place DR genuinely halves PE work since
    the m=1024 contraction needed 8 accumulating bf16 matmuls): lhsT =
    vT~ pairs [128, 2, 65] e4m3 with slot stride 80 (pair step %16==0
    LDWEIGHTS rule), col 64 = ones so accumulator row 64 is the softmax
    denominator; rhs = two m-strips of p (e5m2) per instruction.
  - exp runs on ACT with bias -2.5 writing e5m2 directly (smax ~= 12 ->
    max p ~= 1.3e4 < 57344; row-max min ~= -1.2 keeps every row normal).
    Measured end-to-end rel err ~4e-4 (the residual add uses f32 x).
  - epilogue: gamma folded into woT; denominators DMA-packed [4, 512] ->
    f32 -> reciprocal_approx_fast -> bf16 -> DRAM round-trip broadcast;
    onorm = ou * rden as bf16 tensor_tensor; residual added in the output
    drain (tensor_tensor add with f32 x) instead of an identity matmul.
  - queues: all DMAs on sync; pools stage1 on GPSIMD; ACT only exps.
"""

from contextlib import ExitStack

import numpy as np

import concourse.bacc as bacc
import concourse.mybir as mybir
from concourse import masks
from concourse.alu_op_type import AluOpType
from concourse.tile import TileContext

FP32 = mybir.dt.float32
F32R = mybir.dt.float32r
BF16 = mybir.dt.bfloat16
FP8 = mybir.dt.float8e4      # e4m3
FP8W = mybir.dt.float8e5     # e5m2 (attention weights)
AFT = mybir.ActivationFunctionType
DR = mybir.MatmulPerfMode.DoubleRow

# Per-core problem shape (hardcoded; harness provides full inputs).
B_FULL, C, H, W = 16, 128, 64, 64
N_CORES = 8
B_LOC = B_FULL // N_CORES            # 2
HW = H * W                           # 4096
M = HW // 4                          # 1024 (after 2x2 maxpool)
CQ, CV = C // 8, C // 2              # 16, 64
NCH = 512                            # epilogue chunk (psum-bank n span)
NCHUNKS = HW // NCH                  # 8
NSP = 1024                           # score span (moving operand width)
NSPANS = HW // NSP                   # 4
MT = M // 128                        # 8 m-strips of 128

EXP_BIAS = -2.5                      # exp(s + EXP_BIAS) fits e5m2
VTS = 80                             # vT~ pair slot stride (16-aligned)
WARMUP_MM = 20                       # HAM warmup matmuls


def build_nc():
    nc = bacc.Bacc()
    x_e = nc.declare_dram_parameter("x", [B_LOC, C, HW], FP32, isOutput=False)
    wq_e = nc.declare_dram_parameter("wq", [CQ, C], FP32, isOutput=False)
    wk_e = nc.declare_dram_parameter("wk", [CQ, C], FP32, isOutput=False)
    wv_e = nc.declare_dram_parameter("wv", [CV, C], FP32, isOutput=False)
    wo_e = nc.declare_dram_parameter("wo", [C, CV], FP32, isOutput=False)
    g_e = nc.declare_dram_parameter("gamma", [1], FP32, isOutput=False)
    out_e = nc.declare_dram_parameter("out", [B_LOC, C, HW], FP32, isOutput=True)

    with TileContext(nc) as tc, ExitStack() as ctx:
        const = ctx.enter_context(tc.tile_pool(name="const", bufs=1))
        xpool = ctx.enter_context(tc.tile_pool(name="x", bufs=2))
        qkv = ctx.enter_context(tc.tile_pool(name="qkv", bufs=2))
        ppool = ctx.enter_context(tc.tile_pool(name="p", bufs=4))
        vtpool = ctx.enter_context(tc.tile_pool(name="vt", bufs=8))
        eppool = ctx.enter_context(tc.tile_pool(name="ep", bufs=3))
        outpool = ctx.enter_context(tc.tile_pool(name="outp", bufs=3))
        # PSUM budget (8 banks): scores 2 tags x 1 buf x 2 banks + av 2x1
        # + wm 2x1
        ps_s = ctx.enter_context(tc.tile_pool(name="ps_s", bufs=1, space="PSUM"))
        ps_av = ctx.enter_context(tc.tile_pool(name="ps_av", bufs=2, space="PSUM"))
        ps_w = ctx.enter_context(tc.tile_pool(name="ps_w", bufs=2, space="PSUM"))
        dscratch = ctx.enter_context(tc.tile_pool(name="dscr", bufs=4, space="DRAM"))

        # ---------------- constants / weight preprocessing ----------------
        ident = const.tile([128, 128], FP32)
        masks.make_identity(nc, ident[:])
        ident_bf = const.tile([128, 128], BF16)
        masks.make_identity(nc, ident_bf[:])

        wq_sb = const.tile([CQ, C], FP32, tag="wq")
        wk_sb = const.tile([CQ, C], FP32, tag="wk")
        wv_sb = const.tile([CV, C], FP32, tag="wv")
        wo_sb = const.tile([C, CV], FP32, tag="wo")
        nc.sync.dma_start(wq_sb[:], wq_e[:])
        nc.sync.dma_start(wk_sb[:], wk_e[:])
        nc.sync.dma_start(wv_sb[:], wv_e[:])
        nc.sync.dma_start(wo_sb[:], wo_e[:])

        # gamma broadcast to all 128 partitions: [128, 1]
        g_sb = const.tile([128, 1], FP32, tag="g")
        nc.sync.dma_start(
            g_sb[:, 0:1], g_e[:].unsqueeze(0).partition_broadcast(128)
        )

        # exp bias as an explicit per-partition scalar
        ebias = const.tile([128, 1], FP32, tag="ebias")
        nc.vector.memset(ebias[:], EXP_BIAS)

        # heater source for HAM full-array keep-warm matmuls
        heat_src = const.tile([128, NCH], BF16, tag="heat")
        nc.vector.memset(heat_src[:], 0.0)

        def heater(tag, n=4):
            # full-array matmuls on constant data: count as PE-busy for the
            # HAM activity monitor so the clock stays at 2.4GHz through the
            # tiled score packs (which do not count)
            hp = ps_w.tile([128, NCH], FP32, tag="wm", name=f"heat_{tag}")
            for hi in range(n):
                nc.tensor.matmul(
                    hp[:], ident_bf[:], heat_src[:], start=True, stop=True
                )

        # W_cat^T: cols 0:16 = wq^T, 32:48 = wk^T, 64:128 = wv^T
        ps_wt = ps_w.tile([128, NCH], FP32, tag="wm")
        nc.tensor.transpose(ps_wt[:, 0:CQ], wq_sb[:], ident[0:CQ, 0:CQ])
        nc.tensor.transpose(ps_wt[:, 32 : 32 + CQ], wk_sb[:], ident[0:CQ, 0:CQ])
        nc.tensor.transpose(ps_wt[:, 64 : 64 + CV], wv_sb[:], ident[0:CV, 0:CV])
        wcatT = const.tile([128, 128], BF16, tag="wcatT")
        nc.vector.memset(wcatT[:], 0.0)
        nc.vector.tensor_copy(wcatT[:, 0:CQ], ps_wt[:, 0:CQ])
        nc.vector.tensor_copy(wcatT[:, 32 : 32 + CQ], ps_wt[:, 32 : 32 + CQ])
        nc.vector.tensor_copy(wcatT[:, 64 : 64 + CV], ps_wt[:, 64 : 64 + CV])

        # woT [64, 128] bf16 with gamma folded in
        wog = const.tile([C, CV], FP32, tag="wog")
        nc.vector.tensor_scalar_mul(wog[:], wo_sb[:], g_sb[:, 0:1])
        ps_wo = ps_w.tile([128, NCH], FP32, tag="wm")
        nc.tensor.transpose(ps_wo[0:CV, 0:C], wog[:], ident[:])
        woT = const.tile([CV + 1, C], BF16, tag="woT")
        nc.vector.tensor_copy(woT[0:CV, :], ps_wo[0:CV, 0:C])
        nc.vector.memset(woT[CV : CV + 1, :], 0.0)

        # ---------------- per-batch prep ----------------

        def load_x(b, x_sb=None, chunks=range(NCHUNKS)):
            if x_sb is None:
                x_sb = xpool.tile([C, HW], FP32, tag="x", name=f"x_{b}")
            for cc in chunks:
                csl = slice(cc * NCH, (cc + 1) * NCH)
                nc.sync.dma_start(x_sb[:, csl], x_e[b, :, csl])
            return x_sb

        def prep_init(b, x_sb):
            return {
                "b": b,
                "x_sb": x_sb,
                "x_bf": qkv.tile([C, HW], BF16, tag="xbf", name=f"xbf_{b}"),
                "qv_full": qkv.tile([C, HW], BF16, tag="qvfull", name=f"qf_{b}"),
                "kv_sb": qkv.tile([128, M], BF16, tag="k", name=f"kv_{b}"),
                # q / pooled-k replicated to partitions {0,32,64,96}+0:16 so
                # 4 m-strips of scores run concurrently as 4 PE row groups
                "qrep": qkv.tile([128, HW], BF16, tag="qrep", name=f"qr_{b}"),
                "krep": qkv.tile([128, M], BF16, tag="krep", name=f"kr_{b}"),
                "vt8": [None] * 4,
            }

        def pool_rows(st, cc, lo, hi, eng=None):
            qv_full, kv_sb = st["qv_full"], st["kv_sb"]
            b = st["b"]
            if eng is None:
                eng = nc.vector
            sl = slice(cc * NCH, (cc + 1) * NCH)
            kv1 = qkv.tile([128, 4 * 64], BF16, tag="kv1",
                           name=f"kv1_{b}_{cc}_{lo}")
            pp = qv_full[lo:hi, sl].rearrange(
                "p (h2 two w) -> p h2 two w", h2=4, two=2, w=64
            )
            s1 = kv1[lo:hi, :].rearrange("p (h w) -> p h w", h=4, w=64)
            eng.tensor_tensor(
                s1, pp[:, :, 0, :], pp[:, :, 1, :], AluOpType.max
            )
            s1w = kv1[lo:hi, :].rearrange(
                "p (h w2 two) -> p h w2 two", h=4, w2=32, two=2
            )
            s2 = kv_sb[lo:hi, cc * 128 : (cc + 1) * 128].rearrange(
                "p (h w2) -> p h w2", h=4, w2=32
            )
            eng.tensor_tensor(
                s2, s1w[:, :, :, 0], s1w[:, :, :, 1], AluOpType.max
            )

        def prep_chunk(st, cc):
            b = st["b"]
            x_sb, x_bf = st["x_sb"], st["x_bf"]
            qv_full, kv_sb = st["qv_full"], st["kv_sb"]
            sl = slice(cc * NCH, (cc + 1) * NCH)
            head = b == 0 and cc < 4
            # ACT is idle until the first exp; split the head-critical
            # casts of batch 0's first chunks between ACT and DVE
            if head:
                nc.scalar.copy(x_bf[:, sl], x_sb[:, sl])
            else:
                nc.vector.tensor_copy(x_bf[:, sl], x_sb[:, sl])
            ps_p = ps_w.tile([128, NCH], FP32, tag="wm", name=f"pj_{b}_{cc}")
            # single full-array matmul: counts as PE-busy for the HAM clock
            # gate (tiled/col-split matmuls do not), keeping the PE at 2.4GHz
            nc.tensor.matmul(
                ps_p[:], wcatT[:], x_bf[:, sl], start=True, stop=True
            )
            nc.vector.tensor_copy(qv_full[:, sl], ps_p[:])
            # k-pools gate the early scores; batch 0's v-pools + transposes
            # are deferred past the whole k chain
            pool_rows(st, cc, 32, 32 + CQ)
            if b != 0:
                pool_rows(st, cc, 64, 128)
                emit_vt(st, cc)
            # q / pooled-k replication to the 4 row-group partition offsets,
            # once per half-batch (sync queue; x1 loads are deferred so the
            # head replication isn't stuck behind them)
            if cc in (3, 7):
                h = cc // 4
                qsl = slice(h * 4 * NCH, (h + 1) * 4 * NCH)
                msl = slice(h * 512, (h + 1) * 512)
                for gi in range(4):
                    nc.sync.dma_start(
                        st["qrep"][32 * gi : 32 * gi + CQ, qsl],
                        qv_full[0:CQ, qsl],
                    )
                    nc.sync.dma_start(
                        st["krep"][32 * gi : 32 * gi + CQ, msl],
                        kv_sb[32 : 32 + CQ, msl],
                    )
            # head cut: strip-quad 0 of spans 0-1 only needs q chunks 0-3 and
            # the first pooled-k half -- start the score/exp pipeline early
            if b == 0 and cc == 3:
                for esp in range(2):
                    pc = ppool.tile([128, MT * NSP], FP8W, tag="pc",
                                    name=f"pc_0_{esp}")
                    st.setdefault("early_pc", {})[esp] = pc
                    scores_packs(st, esp, pc, quads=(0,))

        def emit_vt(st, j):
            # vT~ pair tiles [128, 2*VTS] e4m3; strip j -> pair j//2, slot
            # j%2 at cols 0:65 / VTS:VTS+65; col 64 & VTS+64 = ones
            b, kv_sb = st["b"], st["kv_sb"]
            u, s = j // 2, j % 2
            ps_t = ps_w.tile([128, 128], BF16, tag="wm", name=f"tp_{b}_{j}")
            nc.tensor.transpose(
                ps_t[:, 0:CV],
                kv_sb[64:128, j * 128 : (j + 1) * 128],
                ident_bf[64:128, 64:128],
            )
            if s == 0:
                st["vt8"][u] = vtpool.tile([128, 2 * VTS], FP8, tag="vt",
                                           name=f"vt_{b}_{u}")
            vt = st["vt8"][u]
            off = s * VTS
            nc.vector.tensor_copy(vt[:, off : off + CV], ps_t[:, 0:CV])
            nc.vector.memset(vt[:, off + CV : off + CV + 1], 1.0)

        def scores_packs(st, sp, pc, quads=(0, 1), fillers=None):
            # 16-tile packs: 4 m-strips x 4 col groups of [K=16, O=32, N=512]
            # matmuls run concurrently in 32x32 PE tiling mode (~547ns per
            # 4 strips at the cold clock). Each pack lands in two 2-bank
            # PSUM tiles (strips 0,1 / 2,3) so the two exp reads pipeline
            # against the next pack's writes.
            b = st["b"]
            qrep, krep = st["qrep"], st["krep"]
            pcw = pc[:].rearrange("p (t n) -> p t n", t=MT, n=NSP)
            for qd in quads:
                for hh in range(2):
                    nsl = slice(sp * NSP + hh * NCH, sp * NSP + (hh + 1) * NCH)
                    halves = [
                        ps_s.tile([128, 2 * NCH], FP32, tag=f"s{z}",
                                  name=f"s_{b}_{sp}_{qd}_{hh}_{z}")
                        for z in range(2)
                    ]
                    for i in range(4):
                        t = 4 * qd + i
                        s_ps = halves[i // 2]
                        ioff = (i % 2) * NCH
                        for j in range(4):
                            nc.tensor.matmul(
                                s_ps[32 * j : 32 * (j + 1),
                                     ioff : ioff + NCH],
                                krep[32 * i : 32 * i + CQ,
                                     t * 128 + 32 * j : t * 128 + 32 * (j + 1)],
                                qrep[32 * i : 32 * i + CQ, nsl],
                                start=True,
                                stop=True,
                                tile_position=(32 * i, 32 * j),
                            )
                    for z in range(2):
                        nc.scalar.activation(
                            pcw[:, 4 * qd + 2 * z : 4 * qd + 2 * z + 2,
                                hh * NCH : (hh + 1) * NCH],
                            halves[z][:].rearrange(
                                "p (i n) -> p i n", i=2, n=NCH
                            ),
                            AFT.Exp,
                            bias=ebias[:, 0:1],
                        )
                    if fillers:
                        for _ in range(2):
                            if fillers:
                                fillers.pop(0)()

        def av_pass(st, ep, c, pc, u):
            # one DoubleRow accumulation pass (m-strips 2u, 2u+1) of chunk c
            b, vt8 = st["b"], st["vt8"]
            h = c % 2
            pcv = pc[:].rearrange("p (a two n) -> p a two n", a=4, two=2, n=NSP)
            if u == 0:
                ep.setdefault("o_ps", {})[c % 2] = ps_av.tile(
                    [CV + 1, NCH], FP32, tag="av", name=f"av_{b}_{c}"
                )
            nc.tensor.matmul(
                ep["o_ps"][c % 2][:],
                _pair(vt8[u], VTS, 0, CV + 1),
                pcv[:, u, :, h * NCH : (h + 1) * NCH],
                start=(u == 0),
                stop=(u == 3),
                perf_mode=DR,
            )

        def av_finish(st, ep, c):
            b = st["b"]
            ou = eppool.tile([CV + 1, NCH], BF16, tag="ou", bufs=9,
                             name=f"ou_{b}_{c}")
            nc.vector.tensor_copy(ou[:], ep["o_ps"][c % 2][:])
            ep["ou"][c % 4] = ou
            if ep.get("last"):
                # final group: per-chunk reciprocal chains (no 4-chunk
                # batching latency, one-hop gpsimd broadcast) so the tail
                # epilogues start as soon as each AV lands
                if "den" not in ep:
                    ep["den"] = eppool.tile([CV + 1, 4 * NCH], BF16, tag="den",
                                            bufs=2, name=f"den_{ep['bh'][0]}_L")
                dsc = eppool.tile([1, NCH], BF16, tag="dstL", bufs=4,
                                  name=f"dstL_{b}_{c}")
                nc.sync.dma_start(dsc[0:1, :], ou[CV : CV + 1, :])
                ds32 = eppool.tile([1, NCH], FP32, tag="ds32L", bufs=4,
                                   name=f"ds32L_{b}_{c}")
                nc.vector.tensor_copy(ds32[:], dsc[:])
                rdn = eppool.tile([1, NCH], FP32, tag="rdnL", bufs=4,
                                  name=f"rdnL_{b}_{c}")
                nc.vector.reciprocal_approx_fast(rdn[:], ds32[:])
                rdnb = eppool.tile([1, NCH], BF16, tag="rdnbL", bufs=4,
                                   name=f"rdnbL_{b}_{c}")
                nc.vector.tensor_copy(rdnb[:], rdn[:])
                nc.gpsimd.partition_broadcast(
                    ep["den"][:, (c % 4) * NCH : (c % 4 + 1) * NCH],
                    rdnb[0:1, :],
                )
            else:
                nc.sync.dma_start(
                    ep["dstage"][c % 4 : c % 4 + 1, :], ou[CV : CV + 1, :]
                )

        def den_chain(ep, lo=0, hi=4):
            # reciprocal + broadcast of dstage rows [lo, hi) (split for the
            # final group so its first chunks' epilogues don't wait on the
            # last chunk's AV). The hi==4 split part uses dedicated base-0
            # tiles (engine ops can't start at partition 3).
            b, h = ep["bh"]
            if "den" not in ep:
                ep["den"] = eppool.tile([CV + 1, 4 * NCH], BF16, tag="den", bufs=2,
                                        name=f"den_{b}_{h}")
            if lo == 0:
                n, src = hi, ep["dstage"]
            else:
                n, src = 1, ep["dstage2"]
            ds32 = eppool.tile([n, NCH], FP32, tag=f"ds32{lo}", bufs=2,
                               name=f"ds32_{b}_{h}_{lo}")
            nc.vector.tensor_copy(ds32[:], src[0:n, :])
            rdn = eppool.tile([n, NCH], FP32, tag=f"rdn{lo}", bufs=2,
                              name=f"rdn_{b}_{h}_{lo}")
            nc.vector.reciprocal_approx_fast(rdn[:], ds32[:])
            rdnb = eppool.tile([n, NCH], BF16, tag=f"rdnb{lo}", bufs=2,
                               name=f"rdnb_{b}_{h}_{lo}")
            nc.vector.tensor_copy(rdnb[:], rdn[:])
            rd4 = dscratch.tile([n, NCH], BF16, tag=f"rd{lo}",
                                name=f"rd4_{b}_{h}_{lo}")
            nc.sync.dma_start(rd4[:], rdnb[:])
            nc.sync.dma_start(
                ep["den"][:, lo * NCH : hi * NCH].rearrange(
                    "p (c n) -> p c n", c=hi - lo, n=NCH
                ),
                rd4[:].partition_broadcast(CV + 1),
            )

        def epilogue_chunk(st, ep, c):
            b, h = ep["bh"]
            x_sb = st["x_sb"]
            sl = slice(c * NCH, (c + 1) * NCH)
            onorm = eppool.tile([CV + 1, NCH], BF16, tag="onorm", bufs=3,
                                name=f"on_{b}_{c}")
            nc.vector.tensor_tensor(
                onorm[:],
                ep["ou"][c % 4][0 : CV + 1, :],
                ep["den"][:, (c % 4) * NCH : (c % 4 + 1) * NCH],
                AluOpType.mult,
            )
            o2_ps = ps_w.tile([128, NCH], FP32, tag="wm", name=f"o2_{b}_{c}")
            # K=65 rounds the PE tile mode up to full-array (HAM-counted);
            # woT row 64 is zero so the junk denominator row contributes 0
            nc.tensor.matmul(
                o2_ps[:], woT[:], onorm[:], start=True, stop=True
            )
            out_sb = outpool.tile([C, NCH], FP32, tag="out", name=f"os_{b}_{c}")
            nc.vector.tensor_tensor(
                out_sb[:], o2_ps[:], x_sb[:, sl], AluOpType.add
            )
            nc.sync.dma_start(out_e[b, :, sl], out_sb[:])

        # ---- emission: 8 span-iterations (4 spans x 2 batches) ----
        x0 = load_x(0)
        x1 = xpool.tile([C, HW], FP32, tag="x", name="x_1")
        st0 = prep_init(0, x0)
        st1 = prep_init(1, x1)
        for cc in range(NCHUNKS):
            prep_chunk(st0, cc)
        # x1 loads issued after b0's prep DMAs so the head qrep/krep
        # replication isn't queued behind them
        load_x(1, x_sb=x1)
        # batch 0's deferred v-pools + vT transposes (needed by AV(b0)
        # from span-iteration 1 onward)
        for cc in range(NCHUNKS):
            pool_rows(st0, cc, 64, 128)
            emit_vt(st0, cc)
        sts = {0: st0, 1: st1}

        av_q = []      # (st, ep, sp, pc) spans awaiting AV
        ep_q = []      # (st, ep, c, min_iter) awaiting epilogue
        eps = {}

        def av_fillers(pst, pep, psp, ppc, it):
            # AV for span psp as filler units: 4 pass-pairs + drain/den
            fs = []
            for u in range(4):
                def fpass(u=u):
                    av_pass(pst, pep, 2 * psp, ppc, u)
                    av_pass(pst, pep, 2 * psp + 1, ppc, u)
                fs.append(fpass)

            def fin():
                for cch in (2 * psp, 2 * psp + 1):
                    av_finish(pst, pep, cch)
                    if pep.get("last"):
                        ep_q.append((pst, pep, cch, it))
                if not pep.get("last") and (2 * psp + 1) % 4 == 3:
                    den_chain(pep)
                    for ec in range(2 * psp - 2, 2 * psp + 2):
                        ep_q.append((pst, pep, ec, it + 1))
            fs.append(fin)
            return fs

        for i in range(2 * NSPANS):
            b, sp = i // NSPANS, i % NSPANS
            st = sts[b]
            g = (b, sp // 2)
            if g not in eps:
                eps[g] = {
                    "bh": g,
                    "dstage": eppool.tile([4, NCH], BF16, tag="dstage", bufs=3,
                                          name=f"dst_{g[0]}_{g[1]}"),
                    "ou": {},
                    "last": g == (1, 1),
                }
            fillers = []
            if i < NSPANS:
                fillers.append(lambda cc=2 * i: prep_chunk(st1, cc))
                fillers.append(lambda cc=2 * i + 1: prep_chunk(st1, cc))
            if i >= 1 and av_q:
                pst, pep, psp, ppc = av_q.pop(0)
                fillers.extend(av_fillers(pst, pep, psp, ppc, i))
            if b == 0 and sp in st.get("early_pc", {}):
                pc = st["early_pc"][sp]
                quads = (1,)
            else:
                pc = ppool.tile([128, MT * NSP], FP8W, tag="pc",
                                name=f"pc_{b}_{sp}")
                quads = (0, 1)
            if i == 2 * NSPANS - 1:
                # last span: its own AV interleaves into the span instead of
                # serializing into the tail
                fillers.extend(av_fillers(st, eps[g], sp, pc, i))
            scores_packs(st, sp, pc, quads=quads, fillers=fillers)
            for f in fillers:
                f()
            if i != 2 * NSPANS - 1:
                av_q.append((st, eps[g], sp, pc))
            npop = 0
            cap = 3 if i < 6 else 8
            while ep_q and ep_q[0][3] <= i and npop < cap:
                e = ep_q.pop(0)
                epilogue_chunk(e[0], e[1], e[2])
                npop += 1
        # drain remaining work; the final group computed per-chunk
        # denominators in av_finish so each epilogue follows its AV directly
        while av_q:
            pst, pep, psp, ppc = av_q.pop(0)
            for f in av_fillers(pst, pep, psp, ppc, 0):
                f()
            while ep_q:
                e = ep_q.pop(0)
                epilogue_chunk(e[0], e[1], e[2])
        while ep_q:
            e = ep_q.pop(0)
            epilogue_chunk(e[0], e[1], e[2])

    nc.finalize()
    return nc


def _pair(tile, slot, off, n):
    """DoubleRow k-pair view [P, 2, n] of a flat [P, 2*slot] tile: free dim
    is (two, slot) with the window [off:off+n] taken inside each slot."""
    return tile[:].rearrange("p (two f) -> p two f", two=2, f=slot)[
        :, :, off : off + n
    ]


_NC_CACHE = None


def _get_nc():
    global _NC_CACHE
    if _NC_CACHE is None:
        _NC_CACHE = build_nc()
    return _NC_CACHE


def kernel(**inputs) -> np.ndarray:
    from concourse.bass_utils import run_bass_kernel_spmd

    x = np.asarray(inputs["x"], dtype=np.float32).reshape(B_FULL, C, HW)
    wq = np.asarray(inputs["wq"], dtype=np.float32)
    wk = np.asarray(inputs["wk"], dtype=np.float32)
    wv = np.asarray(inputs["wv"], dtype=np.float32)
    wo = np.asarray(inputs["wo"], dtype=np.float32)
    gamma = np.asarray(inputs["gamma"], dtype=np.float32)

    nc = _get_nc()
    in_maps = []
    for i in range(N_CORES):
        in_maps.append(
            {
                "x": np.ascontiguousarray(x[i * B_LOC : (i + 1) * B_LOC]),
                "wq": wq,
                "wk": wk,
                "wv": wv,
                "wo": wo,
                "gamma": gamma,
            }
        )
    res = run_bass_kernel_spmd(nc, in_maps, core_ids=list(range(N_CORES)))
    outs = [res.results[i]["out"].reshape(B_LOC, C, H, W) for i in range(N_CORES)]
    return np.concatenate(outs, axis=0)


if __name__ == "__main__":
    import reference

    inputs = {k: np.asarray(v) for k, v in reference.setup_inputs().items()}
    expected = np.asarray(reference.reference(**inputs))
    actual = kernel(**inputs)
    err = np.linalg.norm(actual - expected) / np.linalg.norm(expected)
    print("Relative error:", err)

